# revision 1
# baseline (speedup 1.0000x reference)
"""Segment-mean (scatter-add + divide) of face features onto vertices, on 8
Trainium2 NeuronCores.

Problem: out[v] = mean over corners c with faces[c]==v of
face_features.reshape(3F, 192)[c], with F=500k faces, V=250k vertices, D=192.

Strategy (window-sharded, no collectives):
  - The vertex space is cut into 128-vertex aligned windows. Host sorts the
    1.5M corner indices by vertex id (index-space metadata only) and assigns
    windows to (core, slot) pairs so that every core's slot s needs the same
    number K_s of 128-corner chunks (sorted dealing of windows by chunk
    count) — the SPMD program is identical across cores while padding stays
    near the ceil(128)-minimum.
  - Corner VALUES are laid out per core in sorted, 128-partition-transposed,
    DMA-contiguous order, split hi/lo into two bf16 halves (val = hi + lo,
    exact up to ~2^-17 relative) with a trailing ones column for on-device
    counts. Both halves flow through the TensorEngine at bf16 rate and
    accumulate into the same fp32 PSUM region, restoring near-fp32 precision
    at 1/4 the PE cost of fp32 matmuls.
  - Per slot, a one-hot matrix [corner, vertex-in-window] is built on the
    Vector engine by comparing each corner's relative vertex id against an
    iota row; the TensorEngine accumulates onehot.T @ [hi|lo][128, 193] into
    PSUM — feature sums in cols 0..191, counts in col 192.
  - DVE computes 1/max(count,1); the Scalar engine applies the scale while
    copying PSUM->SBUF; results are batched per slab and streamed to DRAM.
    Host scatters the window rows back to their vertex positions.

Dummy (padding) corner slots carry relative id -1 so their one-hot row is
zero and they contribute nothing (neither to sums nor counts).
"""

import numpy as np

P = 128          # partitions / window size
D = 192          # feature dim
DC = D + 1       # + count column
NCORES = 8
SLAB_CHUNK_BUDGET = 35   # chunks per DMA slab (~3.3 MB loads)

_prog_cache = {}


def _plan_slabs(ks):
    """Group consecutive slots into slabs of <= SLAB_CHUNK_BUDGET chunks."""
    slabs = []  # (slot_start, n_slots, n_chunks)
    s = 0
    while s < len(ks):
        n_slots = 0
        n_chunks = 0
        while s + n_slots < len(ks) and n_chunks + ks[s + n_slots] <= SLAB_CHUNK_BUDGET:
            n_chunks += ks[s + n_slots]
            n_slots += 1
        if n_slots == 0:  # oversized slot gets its own slab
            n_slots, n_chunks = 1, int(ks[s])
        slabs.append((s, n_slots, n_chunks))
        s += n_slots
    return slabs


def _build_program(ks, mode="bf16hl"):
    import concourse.bacc as bacc
    import concourse.tile as tile
    from concourse import mybir

    nt = len(ks)
    c = int(sum(ks))
    cs = np.concatenate([[0], np.cumsum(ks)]).astype(int)
    slabs = _plan_slabs(ks)
    max_slab_chunks = max(sl[2] for sl in slabs)
    max_slab_slots = max(sl[1] for sl in slabs)
    max_k = int(max(ks))
    f32 = mybir.dt.float32
    bf16 = mybir.dt.bfloat16
    vdt = bf16 if mode == "bf16hl" else f32
    ew = 2 * DC if mode == "bf16hl" else DC

    nc = bacc.Bacc(None, target_bir_lowering=False)
    vals_in = nc.declare_dram_parameter("vals", [P, c * ew], vdt, isOutput=False)
    idxr_in = nc.declare_dram_parameter("idxr", [P, c], vdt, isOutput=False)
    iota_in = nc.declare_dram_parameter("iota", [P, P], vdt, isOutput=False)
    out_dram = nc.declare_dram_parameter("out", [nt * P, D], f32, isOutput=True)
    out_r = out_dram[:].rearrange("(t p) d -> p t d", p=P)

    with tile.TileContext(nc) as tc:
        with (
            tc.tile_pool(name="const", bufs=1) as constp,
            tc.tile_pool(name="slab", bufs=4) as slabp,
            tc.tile_pool(name="oh", bufs=3) as ohp,
            tc.tile_pool(name="small", bufs=6) as smallp,
            tc.tile_pool(name="ot", bufs=3) as otp,
            tc.tile_pool(name="ps", bufs=4, space="PSUM") as psump,
        ):
            iota_t = constp.tile([P, P], vdt)
            nc.sync.dma_start(out=iota_t[:], in_=iota_in[:])
            idxr_t = constp.tile([P, c], vdt)
            nc.sync.dma_start(out=idxr_t[:], in_=idxr_in[:])

            for si, (s0, n_slots, n_chunks) in enumerate(slabs):
                base_chunk = int(cs[s0])
                slab = slabp.tile([P, max_slab_chunks * ew], vdt, tag="slab")
                ldeng = nc.sync if si % 2 == 0 else nc.scalar
                ldeng.dma_start(
                    out=slab[:, : n_chunks * ew],
                    in_=vals_in[
                        :, base_chunk * ew : (base_chunk + n_chunks) * ew
                    ],
                )
                oslab = otp.tile([P, max_slab_slots, D], f32, tag="ot")
                for tt in range(n_slots):
                    t = s0 + tt
                    k_s = int(ks[t])
                    c0 = int(cs[t])       # global chunk index of slot start
                    l0 = c0 - base_chunk  # chunk offset within slab
                    oh = ohp.tile([P, max_k, P], vdt, tag="oh")
                    nc.vector.tensor_tensor(
                        out=oh[:, :k_s, :],
                        in0=idxr_t[:, c0 : c0 + k_s, None].to_broadcast(
                            [P, k_s, P]
                        ),
                        in1=iota_t[:, None, :].to_broadcast([P, k_s, P]),
                        op=mybir.AluOpType.is_equal,
                    )
                    ps = psump.tile([P, DC], f32)
                    nmm = 2 * k_s if mode == "bf16hl" else k_s
                    for k in range(nmm):
                        cc = l0 + (k // 2 if mode == "bf16hl" else k)
                        if mode == "bf16hl":
                            off = cc * ew + (k % 2) * DC
                            lhs = oh[:, k // 2, :]
                        else:
                            off = cc * ew
                            lhs = oh[:, k, :]
                        nc.tensor.matmul(
                            out=ps[:],
                            lhsT=lhs,
                            rhs=slab[:, off : off + DC],
                            start=(k == 0),
                            stop=(k == nmm - 1),
                        )
                    cnt = smallp.tile([P, 1], f32)
                    nc.vector.tensor_scalar_max(cnt[:], ps[:, D:DC], 1.0)
                    rec = smallp.tile([P, 1], f32)
                    nc.vector.reciprocal(rec[:], cnt[:])
                    nc.scalar.activation(
                        out=oslab[:, tt, :],
                        in_=ps[:, 0:D],
                        func=mybir.ActivationFunctionType.Copy,
                        scale=rec[:],
                    )
                # stores ride the ACT HWDGE ring so they don't FIFO-block the
                # slab loads on the SP ring
                nc.scalar.dma_start(
                    out=out_r[:, s0 : s0 + n_slots, :],
                    in_=oslab[:, :n_slots, :],
                )
    nc.compile()
    return nc


def get_program(ks, mode="bf16hl"):
    key = (tuple(int(k) for k in ks), mode)
    if key not in _prog_cache:
        _prog_cache[key] = _build_program(list(key[0]), mode)
    return _prog_cache[key]


def _plan(idx, vcount):
    """Window -> (core, slot) assignment with per-slot uniform chunk counts."""
    nwin_real = (vcount + P - 1) // P
    nwin = -(-nwin_real // NCORES) * NCORES  # pad to multiple of NCORES
    nt = nwin // NCORES
    counts = np.bincount(idx, minlength=nwin * P)
    win_w = counts.reshape(nwin, P).sum(1)
    cw = np.maximum((win_w + P - 1) // P, 1).astype(np.int64)
    o = np.argsort(-cw, kind="stable")
    groups = o.reshape(nt, NCORES)      # groups[s, j] = window id
    ks = cw[groups].max(1)              # = cw[groups[:, 0]]
    return groups, ks, counts


def _host_prep(vals_flat, idx, groups, ks, mode="bf16hl"):
    import ml_dtypes

    bf16 = ml_dtypes.bfloat16
    nt = groups.shape[0]
    nwin = nt * NCORES
    c = int(ks.sum())
    cs = np.concatenate([[0], np.cumsum(ks)]).astype(np.int64)
    ndt = bf16 if mode == "bf16hl" else np.float32

    # sorted corner stream
    order = np.argsort(idx, kind="stable")
    idx_s = idx[order]
    wod = idx_s >> 7                                  # window of each corner
    win_start = np.searchsorted(idx_s, np.arange(nwin, dtype=np.int64) * P)
    pos_in_win = np.arange(len(idx_s), dtype=np.int64) - win_start[wod]

    # window -> (core, slot)
    slot_of = np.empty(nwin, dtype=np.int64)
    core_of = np.empty(nwin, dtype=np.int64)
    for j in range(NCORES):
        slot_of[groups[:, j]] = np.arange(nt)
        core_of[groups[:, j]] = j

    corner_core = core_of[wod]
    corner_slot = slot_of[wod]
    corner_chunk = cs[corner_slot] + (pos_in_win >> 7)
    corner_part = pos_in_win & (P - 1)
    corner_rel = (idx_s & (P - 1)).astype(ndt)

    iota = np.tile(np.arange(P, dtype=ndt), (P, 1))
    in_maps = []
    for j in range(NCORES):
        m = corner_core == j
        gmap = np.zeros((P, c), dtype=np.int64)
        idxr = np.full((P, c), -1.0, dtype=ndt)
        gmap[corner_part[m], corner_chunk[m]] = order[m]
        idxr[corner_part[m], corner_chunk[m]] = corner_rel[m]

        g = vals_flat[gmap]  # [P, c, D] f32
        if mode == "bf16hl":
            vals3 = np.zeros((P, c, 2, DC), dtype=bf16)
            hi_v = g.astype(bf16)
            vals3[:, :, 0, :D] = hi_v
            vals3[:, :, 0, D] = bf16(1.0)
            vals3[:, :, 1, :D] = (g - hi_v.astype(np.float32)).astype(bf16)
            vals2 = vals3.reshape(P, c * 2 * DC)
        else:
            vals3 = np.empty((P, c, DC), dtype=np.float32)
            vals3[:, :, :D] = g
            vals3[:, :, D] = 1.0
            vals2 = vals3.reshape(P, c * DC)
        in_maps.append({"vals": vals2, "idxr": idxr, "iota": iota})
    return in_maps


def run(face_features, faces, vertex_count, mode="bf16hl", trace=False, tmpdir=None):
    from concourse.bass_utils import run_bass_kernel_spmd

    vcount = int(vertex_count)
    ff = np.ascontiguousarray(np.asarray(face_features, dtype=np.float32))
    nf = ff.shape[0]
    vals_flat = ff.reshape(nf * 3, D)
    idx = np.asarray(faces).reshape(-1).astype(np.int64)
    assert idx.min() >= 0 and idx.max() < vcount, "face indices out of range"

    groups, ks, _ = _plan(idx, vcount)
    nc = get_program(ks)
    in_maps = _host_prep(vals_flat, idx, groups, ks, mode=mode)
    kw = {}
    if trace:
        kw = dict(trace=True, tmpdir=tmpdir)
    res = run_bass_kernel_spmd(nc, in_maps, list(range(NCORES)), **kw)

    nt = groups.shape[0]
    nwin = nt * NCORES
    out = np.empty((nwin * P, D), dtype=np.float32)
    out_w = out.reshape(nwin, P, D)
    for j in range(NCORES):
        out_w[groups[:, j]] = res.results[j]["out"].reshape(nt, P, D)
    return out[:vcount], res


def kernel(face_features, faces, vertex_count):
    out, _ = run(face_features, faces, vertex_count)
    return out



# revision 9
# speedup vs baseline: 1.9385x; 1.9385x over previous
"""Segment-mean (scatter-add + divide) of face features onto vertices, on 8
Trainium2 NeuronCores.

Problem: out[v] = mean over corners c with faces[c]==v of
face_features.reshape(3F, 192)[c], with F=500k faces, V=250k vertices, D=192.

Strategy (window-sharded, no collectives, DMA-roofline oriented):
  - The vertex space is cut into 128-vertex aligned windows. Host sorts the
    1.5M corner indices by vertex id (index-space metadata only) and assigns
    windows to (core, slot) pairs so that every core's slot s needs the same
    number K_s of 128-corner chunks (sorted dealing of windows by chunk
    count) — the SPMD program is identical across cores while padding stays
    near the ceil(128)-minimum.
  - Corner VALUES are laid out per core in sorted, 128-partition-transposed,
    DMA-contiguous order as plain bf16 (~0.2% rounding, 10x inside the 2e-2
    gate) — half the HBM traffic of an fp32-accurate encoding, and the
    matmuls run at full bf16 PE rate.
  - Per chunk, a one-hot matrix [corner, vertex-in-window] is built on the
    Vector engine via tensor_scalar(is_equal) — iota row as the packed bf16
    in0, the chunk's relative vertex ids as a per-partition fp32 scalar —
    which qualifies for the DVE 4x_2p perf mode (4 elem/cycle/lane), unlike
    a broadcast tensor_tensor which runs 1x. The TensorEngine accumulates
    onehot.T @ vals[128, 192] into PSUM.
  - Per-vertex reciprocal hit counts are computed on the host (they are a
    byproduct of planning) and shipped as a tiny [P, nt] fp32 tensor; the
    Scalar engine applies them while copying PSUM->SBUF, casting to bf16.
  - Results are batched per slab in slot-contiguous [P, slots*D] layout and
    streamed to DRAM as large per-partition-contiguous stores; the host
    transposes/casts back to the full fp32 output.

Dummy (padding) corner slots carry relative id -1 so their one-hot row is
zero and they contribute nothing.
"""

import numpy as np

P = 128          # partitions / window size
D = 192          # feature dim
NCORES = 8
SLAB_CHUNK_BUDGET = 96   # chunks per DMA slab (~4.7 MB loads)

_prog_cache = {}


def _plan_slabs(ks):
    """Group consecutive slots into slabs of <= SLAB_CHUNK_BUDGET chunks."""
    slabs = []  # (slot_start, n_slots, n_chunks)
    s = 0
    while s < len(ks):
        n_slots = 0
        n_chunks = 0
        while s + n_slots < len(ks) and n_chunks + ks[s + n_slots] <= SLAB_CHUNK_BUDGET:
            n_chunks += ks[s + n_slots]
            n_slots += 1
        if n_slots == 0:  # oversized slot gets its own slab
            n_slots, n_chunks = 1, int(ks[s])
        slabs.append((s, n_slots, n_chunks))
        s += n_slots
    return slabs


def _build_program(ks):
    import concourse.bacc as bacc
    import concourse.tile as tile
    from concourse import mybir

    nt = len(ks)
    c = int(sum(ks))
    cs = np.concatenate([[0], np.cumsum(ks)]).astype(int)
    slabs = _plan_slabs(ks)
    max_slab_chunks = max(sl[2] for sl in slabs)
    max_slab_slots = max(sl[1] for sl in slabs)
    max_k = int(max(ks))
    f32 = mybir.dt.float32
    bf16 = mybir.dt.bfloat16

    nc = bacc.Bacc(None, target_bir_lowering=False)
    vals_in = nc.declare_dram_parameter("vals", [P, c * D], bf16, isOutput=False)
    idxr_in = nc.declare_dram_parameter("idxr", [P, c], f32, isOutput=False)
    iota_in = nc.declare_dram_parameter("iota", [P, P], bf16, isOutput=False)
    rec_in = nc.declare_dram_parameter("rec", [P, nt], f32, isOutput=False)
    out_dram = nc.declare_dram_parameter("out", [P, nt * D], bf16, isOutput=True)

    with tile.TileContext(nc) as tc:
        with (
            tc.tile_pool(name="const", bufs=1) as constp,
            tc.tile_pool(name="slab", bufs=3) as slabp,
            tc.tile_pool(name="oh", bufs=4) as ohp,
            tc.tile_pool(name="ot", bufs=3) as otp,
            tc.tile_pool(name="ps", bufs=8, space="PSUM") as psump,
        ):
            iota_t = constp.tile([P, P], bf16)
            nc.sync.dma_start(out=iota_t[:], in_=iota_in[:])
            rec_t = constp.tile([P, nt], f32)
            nc.sync.dma_start(out=rec_t[:], in_=rec_in[:])
            idxr_t = constp.tile([P, c], f32)
            nc.sync.dma_start(out=idxr_t[:], in_=idxr_in[:])

            for si, (s0, n_slots, n_chunks) in enumerate(slabs):
                base_chunk = int(cs[s0])
                slab = slabp.tile([P, max_slab_chunks * D], bf16, tag="slab")
                ldeng = nc.sync if si % 2 == 0 else nc.scalar
                ldeng.dma_start(
                    out=slab[:, : n_chunks * D],
                    in_=vals_in[:, base_chunk * D : (base_chunk + n_chunks) * D],
                )
                oslab = otp.tile([P, max_slab_slots * D], bf16, tag="ot")
                for tt in range(n_slots):
                    t = s0 + tt
                    k_s = int(ks[t])
                    c0 = int(cs[t])       # global chunk index of slot start
                    l0 = c0 - base_chunk  # chunk offset within slab
                    oh = ohp.tile([P, max_k, P], bf16, tag="oh")
                    for k in range(k_s):
                        nc.vector.tensor_scalar(
                            out=oh[:, k, :],
                            in0=iota_t[:],
                            scalar1=idxr_t[:, c0 + k : c0 + k + 1],
                            scalar2=None,
                            op0=mybir.AluOpType.is_equal,
                        )
                    ps = psump.tile([P, D], f32)
                    for k in range(k_s):
                        off = (l0 + k) * D
                        nc.tensor.matmul(
                            out=ps[:],
                            lhsT=oh[:, k, :],
                            rhs=slab[:, off : off + D],
                            start=(k == 0),
                            stop=(k == k_s - 1),
                        )
                    nc.scalar.activation(
                        out=oslab[:, tt * D : (tt + 1) * D],
                        in_=ps[:],
                        func=mybir.ActivationFunctionType.Copy,
                        scale=rec_t[:, t : t + 1],
                    )
                # slot-contiguous per-partition store rides the Pool SWDGE
                # ring so it doesn't FIFO-block slab loads on the SP/ACT rings
                nc.gpsimd.dma_start(
                    out=out_dram[:, s0 * D : (s0 + n_slots) * D],
                    in_=oslab[:, : n_slots * D],
                )
    nc.compile()
    return nc


def get_program(ks):
    key = tuple(int(k) for k in ks)
    if key not in _prog_cache:
        _prog_cache[key] = _build_program(list(key))
    return _prog_cache[key]


def _plan(idx, vcount):
    """Window -> (core, slot) assignment with per-slot uniform chunk counts."""
    nwin_real = (vcount + P - 1) // P
    nwin = -(-nwin_real // NCORES) * NCORES  # pad to multiple of NCORES
    nt = nwin // NCORES
    counts = np.bincount(idx, minlength=nwin * P)
    win_w = counts.reshape(nwin, P).sum(1)
    cw = np.maximum((win_w + P - 1) // P, 1).astype(np.int64)
    o = np.argsort(-cw, kind="stable")
    groups = o.reshape(nt, NCORES)      # groups[s, j] = window id
    ks = cw[groups].max(1)              # = cw[groups[:, 0]]
    return groups, ks, counts


def _host_prep(vals_flat, idx, groups, ks, counts):
    import ml_dtypes

    bf16 = ml_dtypes.bfloat16
    nt = groups.shape[0]
    nwin = nt * NCORES
    c = int(ks.sum())
    cs = np.concatenate([[0], np.cumsum(ks)]).astype(np.int64)

    # sorted corner stream
    order = np.argsort(idx, kind="stable")
    idx_s = idx[order]
    wod = idx_s >> 7                                  # window of each corner
    win_start = np.searchsorted(idx_s, np.arange(nwin, dtype=np.int64) * P)
    pos_in_win = np.arange(len(idx_s), dtype=np.int64) - win_start[wod]

    # window -> (core, slot)
    slot_of = np.empty(nwin, dtype=np.int64)
    core_of = np.empty(nwin, dtype=np.int64)
    for j in range(NCORES):
        slot_of[groups[:, j]] = np.arange(nt)
        core_of[groups[:, j]] = j

    corner_core = core_of[wod]
    corner_slot = slot_of[wod]
    corner_chunk = cs[corner_slot] + (pos_in_win >> 7)
    corner_part = pos_in_win & (P - 1)
    corner_rel = (idx_s & (P - 1)).astype(np.float32)

    recip = (1.0 / np.maximum(counts, 1)).astype(np.float32).reshape(nwin, P)

    iota = np.tile(np.arange(P, dtype=bf16), (P, 1))
    in_maps = []
    for j in range(NCORES):
        m = corner_core == j
        gmap = np.zeros((P, c), dtype=np.int64)
        idxr = np.full((P, c), -1.0, dtype=np.float32)
        gmap[corner_part[m], corner_chunk[m]] = order[m]
        idxr[corner_part[m], corner_chunk[m]] = corner_rel[m]

        vals2 = vals_flat[gmap].astype(bf16).reshape(P, c * D)
        rec = np.ascontiguousarray(recip[groups[:, j]].T)  # [P, nt]
        in_maps.append({"vals": vals2, "idxr": idxr, "iota": iota, "rec": rec})
    return in_maps


def run(face_features, faces, vertex_count, trace=False, tmpdir=None):
    from concourse.bass_utils import run_bass_kernel_spmd

    vcount = int(vertex_count)
    ff = np.ascontiguousarray(np.asarray(face_features, dtype=np.float32))
    nf = ff.shape[0]
    vals_flat = ff.reshape(nf * 3, D)
    idx = np.asarray(faces).reshape(-1).astype(np.int64)
    assert idx.min() >= 0 and idx.max() < vcount, "face indices out of range"

    groups, ks, counts = _plan(idx, vcount)
    nc = get_program(ks)
    in_maps = _host_prep(vals_flat, idx, groups, ks, counts)
    kw = {}
    if trace:
        kw = dict(trace=True, tmpdir=tmpdir)
    res = run_bass_kernel_spmd(nc, in_maps, list(range(NCORES)), **kw)

    nt = groups.shape[0]
    nwin = nt * NCORES
    out = np.empty((nwin * P, D), dtype=np.float32)
    out_w = out.reshape(nwin, P, D)
    for j in range(NCORES):
        r = np.asarray(res.results[j]["out"]).reshape(P, nt, D)
        out_w[groups[:, j]] = r.transpose(1, 0, 2).astype(np.float32)
    return out[:vcount], res


def kernel(face_features, faces, vertex_count):
    out, _ = run(face_features, faces, vertex_count)
    return out


# revision 15
# speedup vs baseline: 2.1394x; 1.1036x over previous
"""Segment-mean (scatter-add + divide) of face features onto vertices, on 8
Trainium2 NeuronCores.

Problem: out[v] = mean over corners c with faces[c]==v of
face_features.reshape(3F, 192)[c], with F=500k faces, V=250k vertices, D=192.

Strategy (window-sharded, no collectives, DMA-roofline oriented):
  - The vertex space is cut into 128-vertex aligned windows. Host sorts the
    1.5M corner indices by vertex id (index-space metadata only) and assigns
    windows to (core, slot) pairs so that every core's slot s needs the same
    number K_s of 128-corner chunks (sorted dealing of windows by chunk
    count) — the SPMD program is identical across cores while padding stays
    near the ceil(128)-minimum.
  - Corner VALUES are laid out per core in sorted, 128-partition-transposed,
    DMA-contiguous order as plain bf16 (~0.2% rounding, 10x inside the 2e-2
    gate) — half the HBM traffic of an fp32-accurate encoding, and the
    matmuls run at full bf16 PE rate.
  - Per slot, a one-hot matrix [corner, vertex-in-window] is built either on
    the Vector engine (per-chunk tensor_scalar(is_equal): iota row vs the
    chunk's relative vertex ids as a per-partition fp32 scalar) or on the
    GpSimd engine (one local_scatter of a ones-row into the slot's whole
    [128, k*128] one-hot using pre-offset int16 indices, negatives ignored).
    Slots are dealt greedily to the two engines so neither is a bottleneck.
    The TensorEngine accumulates onehot.T @ vals[128, 192] into PSUM.
  - Per-vertex reciprocal hit counts are computed on the host (they are a
    byproduct of planning) and shipped as a tiny [P, nt] fp32 tensor; the
    Scalar engine applies them while copying PSUM->SBUF, casting to bf16.
  - Results are batched per slab in slot-contiguous [P, slots*D] layout and
    streamed to DRAM as large per-partition-contiguous stores; the host
    transposes/casts back to the full fp32 output.

Dummy (padding) corner slots carry relative id -1 so their one-hot row is
zero and they contribute nothing.
"""

import numpy as np

P = 128          # partitions / window size
D = 192          # feature dim
NCORES = 8
SLAB_CHUNK_BUDGET = 96   # chunks per steady-state DMA slab (~4.7 MB loads)
SLAB_RAMP = (16, 32, 64)  # smaller first slabs so compute starts early

_prog_cache = {}


def _slot_k2(k):
    """local_scatter needs an even index count per slot."""
    return k + (k & 1)


def _plan_slabs(ks):
    """Group consecutive slots into slabs; first slabs are smaller so the
    pipeline fills quickly."""
    slabs = []  # (slot_start, n_slots, n_chunks)
    s = 0
    while s < len(ks):
        budget = SLAB_RAMP[len(slabs)] if len(slabs) < len(SLAB_RAMP) else SLAB_CHUNK_BUDGET
        n_slots = 0
        n_chunks = 0
        while s + n_slots < len(ks) and n_chunks + ks[s + n_slots] <= budget:
            n_chunks += ks[s + n_slots]
            n_slots += 1
        if n_slots == 0:  # oversized slot gets its own slab
            n_slots, n_chunks = 1, int(ks[s])
        slabs.append((s, n_slots, n_chunks))
        s += n_slots
    return slabs


def _build_program(ks):
    import concourse.bacc as bacc
    import concourse.tile as tile
    from concourse import mybir

    nt = len(ks)
    c = int(sum(ks))
    cs = np.concatenate([[0], np.cumsum(ks)]).astype(int)
    k2s = [_slot_k2(int(k)) for k in ks]
    c2 = int(sum(k2s))
    cs2 = np.concatenate([[0], np.cumsum(k2s)]).astype(int)
    slabs = _plan_slabs(ks)
    max_slab_chunks = max(sl[2] for sl in slabs)
    max_slab_slots = max(sl[1] for sl in slabs)
    max_k2 = _slot_k2(int(max(ks)))
    f32 = mybir.dt.float32
    bf16 = mybir.dt.bfloat16
    i16 = mybir.dt.int16

    nc = bacc.Bacc(None, target_bir_lowering=False)
    vals_in = nc.declare_dram_parameter("vals", [P, c * D], bf16, isOutput=False)
    idxr_in = nc.declare_dram_parameter("idxr", [P, c], f32, isOutput=False)
    idxi_in = nc.declare_dram_parameter("idxi", [P, c2], i16, isOutput=False)
    iota_in = nc.declare_dram_parameter("iota", [P, P], bf16, isOutput=False)
    rec_in = nc.declare_dram_parameter("rec", [P, nt], f32, isOutput=False)
    out_dram = nc.declare_dram_parameter("out", [P, nt * D], bf16, isOutput=True)

    # greedy deal of one-hot builds between DVE (tensor_scalar is_equal,
    # ~163 ns/chunk) and Pool (local_scatter, est ~180 ns/chunk + launch)
    acc_v = acc_g = 0.0

    with tile.TileContext(nc) as tc:
        with (
            tc.tile_pool(name="const", bufs=1) as constp,
            tc.tile_pool(name="slab", bufs=3) as slabp,
            tc.tile_pool(name="oh", bufs=4) as ohp,
            tc.tile_pool(name="ot", bufs=3) as otp,
            tc.tile_pool(name="ps", bufs=8, space="PSUM") as psump,
        ):
            iota_t = constp.tile([P, P], bf16)
            nc.scalar.dma_start(out=iota_t[:], in_=iota_in[:])
            rec_t = constp.tile([P, nt], f32)
            nc.scalar.dma_start(out=rec_t[:], in_=rec_in[:])
            idxr_t = constp.tile([P, c], f32)
            nc.scalar.dma_start(out=idxr_t[:], in_=idxr_in[:])
            idxi_t = constp.tile([P, c2], i16)
            nc.scalar.dma_start(out=idxi_t[:], in_=idxi_in[:])
            ones_t = constp.tile([P, max_k2], bf16)
            nc.gpsimd.memset(ones_t[:], 1.0)

            for si, (s0, n_slots, n_chunks) in enumerate(slabs):
                base_chunk = int(cs[s0])
                slab = slabp.tile([P, max_slab_chunks * D], bf16, tag="slab")
                ldeng = nc.sync if si % 2 == 0 else nc.scalar
                ldeng.dma_start(
                    out=slab[:, : n_chunks * D],
                    in_=vals_in[:, base_chunk * D : (base_chunk + n_chunks) * D],
                )
                oslab = otp.tile([P, max_slab_slots * D], bf16, tag="ot")
                for tt in range(n_slots):
                    t = s0 + tt
                    k_s = int(ks[t])
                    k2 = k2s[t]
                    c0 = int(cs[t])       # global chunk index of slot start
                    l0 = c0 - base_chunk  # chunk offset within slab
                    oh = ohp.tile([P, max_k2, P], bf16, tag="oh")
                    cost_v = 163.0 * k_s + 40.0
                    cost_g = 180.0 * k2 + 250.0
                    if acc_v + cost_v <= acc_g + cost_g:
                        acc_v += cost_v
                        for k in range(k_s):
                            nc.vector.tensor_scalar(
                                out=oh[:, k, :],
                                in0=iota_t[:],
                                scalar1=idxr_t[:, c0 + k : c0 + k + 1],
                                scalar2=None,
                                op0=mybir.AluOpType.is_equal,
                            )
                    else:
                        acc_g += cost_g
                        i0 = int(cs2[t])
                        nc.gpsimd.local_scatter(
                            out_ap=oh[:, :k2, :],
                            data_ap=ones_t[:, :k2],
                            idxs_ap=idxi_t[:, i0 : i0 + k2],
                            channels=P,
                            num_elems=k2 * P,
                            num_idxs=k2,
                        )
                    ps = psump.tile([P, D], f32)
                    for k in range(k_s):
                        off = (l0 + k) * D
                        nc.tensor.matmul(
                            out=ps[:],
                            lhsT=oh[:, k, :],
                            rhs=slab[:, off : off + D],
                            start=(k == 0),
                            stop=(k == k_s - 1),
                        )
                    nc.scalar.activation(
                        out=oslab[:, tt * D : (tt + 1) * D],
                        in_=ps[:],
                        func=mybir.ActivationFunctionType.Copy,
                        scale=rec_t[:, t : t + 1],
                    )
                # slot-contiguous per-partition store rides the Pool SWDGE
                # ring so it doesn't FIFO-block slab loads on the SP/ACT rings
                nc.gpsimd.dma_start(
                    out=out_dram[:, s0 * D : (s0 + n_slots) * D],
                    in_=oslab[:, : n_slots * D],
                )
    nc.compile()
    return nc


def get_program(ks):
    key = tuple(int(k) for k in ks)
    if key not in _prog_cache:
        _prog_cache[key] = _build_program(list(key))
    return _prog_cache[key]


def _plan(idx, vcount):
    """Window -> (core, slot) assignment with per-slot uniform chunk counts."""
    nwin_real = (vcount + P - 1) // P
    nwin = -(-nwin_real // NCORES) * NCORES  # pad to multiple of NCORES
    nt = nwin // NCORES
    counts = np.bincount(idx, minlength=nwin * P)
    win_w = counts.reshape(nwin, P).sum(1)
    cw = np.maximum((win_w + P - 1) // P, 1).astype(np.int64)
    o = np.argsort(-cw, kind="stable")
    groups = o.reshape(nt, NCORES)      # groups[s, j] = window id
    ks = cw[groups].max(1)              # = cw[groups[:, 0]]
    return groups, ks, counts


def _host_prep(vals_flat, idx, groups, ks, counts):
    import ml_dtypes

    bf16 = ml_dtypes.bfloat16
    nt = groups.shape[0]
    nwin = nt * NCORES
    c = int(ks.sum())
    cs = np.concatenate([[0], np.cumsum(ks)]).astype(np.int64)
    k2s = ks + (ks & 1)
    c2 = int(k2s.sum())
    cs2 = np.concatenate([[0], np.cumsum(k2s)]).astype(np.int64)

    # sorted corner stream
    order = np.argsort(idx, kind="stable")
    idx_s = idx[order]
    wod = idx_s >> 7                                  # window of each corner
    win_start = np.searchsorted(idx_s, np.arange(nwin, dtype=np.int64) * P)
    pos_in_win = np.arange(len(idx_s), dtype=np.int64) - win_start[wod]

    # window -> (core, slot)
    slot_of = np.empty(nwin, dtype=np.int64)
    core_of = np.empty(nwin, dtype=np.int64)
    for j in range(NCORES):
        slot_of[groups[:, j]] = np.arange(nt)
        core_of[groups[:, j]] = j

    corner_core = core_of[wod]
    corner_slot = slot_of[wod]
    chunk_in_slot = pos_in_win >> 7
    corner_chunk = cs[corner_slot] + chunk_in_slot
    corner_chunk2 = cs2[corner_slot] + chunk_in_slot
    corner_part = pos_in_win & (P - 1)
    corner_rel = (idx_s & (P - 1)).astype(np.float32)
    corner_sidx = (chunk_in_slot * P + (idx_s & (P - 1))).astype(np.int16)

    recip = (1.0 / np.maximum(counts, 1)).astype(np.float32).reshape(nwin, P)

    iota = np.tile(np.arange(P, dtype=bf16), (P, 1))
    in_maps = []
    for j in range(NCORES):
        m = corner_core == j
        gmap = np.zeros((P, c), dtype=np.int64)
        idxr = np.full((P, c), -1.0, dtype=np.float32)
        idxi = np.full((P, c2), -1, dtype=np.int16)
        gmap[corner_part[m], corner_chunk[m]] = order[m]
        idxr[corner_part[m], corner_chunk[m]] = corner_rel[m]
        idxi[corner_part[m], corner_chunk2[m]] = corner_sidx[m]

        vals2 = vals_flat[gmap].astype(bf16).reshape(P, c * D)
        rec = np.ascontiguousarray(recip[groups[:, j]].T)  # [P, nt]
        in_maps.append(
            {"vals": vals2, "idxr": idxr, "idxi": idxi, "iota": iota, "rec": rec}
        )
    return in_maps


def run(face_features, faces, vertex_count, trace=False, tmpdir=None):
    from concourse.bass_utils import run_bass_kernel_spmd

    vcount = int(vertex_count)
    ff = np.ascontiguousarray(np.asarray(face_features, dtype=np.float32))
    nf = ff.shape[0]
    vals_flat = ff.reshape(nf * 3, D)
    idx = np.asarray(faces).reshape(-1).astype(np.int64)
    assert idx.min() >= 0 and idx.max() < vcount, "face indices out of range"

    groups, ks, counts = _plan(idx, vcount)
    nc = get_program(ks)
    in_maps = _host_prep(vals_flat, idx, groups, ks, counts)
    kw = {}
    if trace:
        kw = dict(trace=True, tmpdir=tmpdir)
    res = run_bass_kernel_spmd(nc, in_maps, list(range(NCORES)), **kw)

    nt = groups.shape[0]
    nwin = nt * NCORES
    out = np.empty((nwin * P, D), dtype=np.float32)
    out_w = out.reshape(nwin, P, D)
    for j in range(NCORES):
        r = np.asarray(res.results[j]["out"]).reshape(P, nt, D)
        out_w[groups[:, j]] = r.transpose(1, 0, 2).astype(np.float32)
    return out[:vcount], res


def kernel(face_features, faces, vertex_count):
    out, _ = run(face_features, faces, vertex_count)
    return out
